# revision 15
# baseline (speedup 1.0000x reference)
"""Trainium2 Bass kernel for nn_Actor (ragged GTrXL-style actor network).

Pure data parallel over 8 NeuronCores: 8 samples per core. The full forward
(one-hot voxel featurization via a hinge-basis decomposition, rel-pos
attention with ragged masks, two GRU gates, LN/MLP, voxel-mean head,
value/action heads) runs on-device in a single NEFF; the host only shards
inputs / reassembles outputs.

Self-contained: hardcodes all shapes; no sibling imports.
"""
import sys

if "/opt/trn_rl_repo" not in sys.path:
    sys.path.insert(0, "/opt/trn_rl_repo")

import numpy as np

# ---- problem constants ----
B, T_OBS, GF = 64, 101, 7
T = 100
NCELL, NMAT = 1000, 4
VDIM = NCELL * NMAT          # 4000
D, H, HD = 64, 2, 32
GDIM, VFEAT = 16, 48
TAU = 50
MLP_D = 32
NOUT = 16
EPS = 1e-5
NCORES = 8
BC = B // NCORES             # 8 samples / core
NTOK = BC * T                # 800 tokens / core
FT = TAU + T                 # 150
NFTOK = BC * FT              # 1200
INV_SQRT_HD = 1.0 / np.sqrt(HD)

_CACHE = {}


def _relpos_table():
    inv = 1.0 / (10000.0 ** (np.arange(0, D, 2, dtype=np.float32) / D))
    pos = np.arange(FT - 1, -1, -1, dtype=np.float32)
    ang = pos[:, None] * inv[None, :]
    return np.concatenate([np.sin(ang), np.cos(ang)], -1).T.copy()  # [64, 150]


def _build():
    import concourse.bass as bass
    import concourse.tile as tile
    from concourse import bacc, mybir
    from concourse.masks import make_identity
    from contextlib import ExitStack

    f32 = mybir.dt.float32
    i8 = mybir.dt.int8
    AF = mybir.ActivationFunctionType
    OP = mybir.AluOpType
    AX = mybir.AxisListType

    nc = bacc.Bacc("TRN2", target_bir_lowering=False, debug=False,
                   enable_asserts=True, num_devices=NCORES)

    def din(name, shape):
        return nc.declare_dram_parameter(name, list(shape), f32, isOutput=False)

    # per-core inputs
    vox = nc.declare_dram_parameter("vox", [NCELL, NTOK], i8, isOutput=False)
    gaus = din("gaus", (GF + 1, NTOK))
    st0 = din("st0", (D, BC * TAU))
    aug2 = din("aug2", (2, NTOK))
    validT = din("validT", (T, BC))
    avmaskT = din("avmaskT", (T, BC))
    sel = din("sel", (1, NTOK))
    # params / constants (replicated)
    wg = din("wg", (GF, GDIM))
    bg = din("bg", (GDIM,))
    wvm = din("wvm", (NMAT, NCELL, VFEAT))
    bv = din("bv", (VFEAT,))
    ln1g = din("ln1g", (D,)); ln1b = din("ln1b", (D,))
    ln2g = din("ln2g", (D,)); ln2b = din("ln2b", (D,))
    wqkv = din("wqkv", (D, 3 * D))
    wpos = din("wpos", (D, D))
    uv = din("uv", (2, D))
    wattn = din("wattn", (D, D))
    g1w = din("g1w", (6, D, D)); g1bz = din("g1bz", (D,))
    g2w = din("g2w", (6, D, D)); g2bz = din("g2bz", (D,))
    we1 = din("we1", (D, MLP_D)); we2 = din("we2", (MLP_D, D))
    wvox = din("wvox", (D, VDIM)); bvox = din("bvox", (VDIM,))
    wa1 = din("wa1", (D, D)); ba1 = din("ba1", (D,))
    wa2 = din("wa2", (D, NOUT)); ba2 = din("ba2", (NOUT,))
    wval1 = din("wval1", (D, D)); bval1 = din("bval1", (D,))
    wval2 = din("wval2", (D, 1)); bval2 = din("bval2", (1,))
    relposT = din("relposT", (D, FT))
    causal = din("causal", (T, FT))
    # outputs
    ovox = nc.declare_dram_parameter("ovox", [NTOK, VDIM], f32, isOutput=True)
    oact = nc.declare_dram_parameter("oact", [NOUT, BC], f32, isOutput=True)
    oval = nc.declare_dram_parameter("oval", [1, BC], f32, isOutput=True)

    KT = [128] * 7 + [104]                          # cell tiles
    CH = [(0, 512), (512, 288)]                     # NTOK chunks (<=512)
    CH3 = [(0, 512), (512, 512), (1024, 176)]       # NFTOK chunks

    with tile.TileContext(nc) as tc, ExitStack() as ctx:
        dma = nc.gpsimd.dma_start

        pconst = ctx.enter_context(tc.tile_pool(name="pconst", bufs=1))
        pbig = ctx.enter_context(tc.tile_pool(name="pbig", bufs=1))

        # ---------- constants / weights into SBUF ----------
        ident = pconst.tile([128, 128], f32)
        make_identity(nc, ident[:])

        _uid = [0]

        def load(src_ap, shape):
            _uid[0] += 1
            t_ = pconst.tile(list(shape), f32, tag=f"ld{_uid[0]}")
            dma(out=t_[:], in_=src_ap)
            return t_

        wg_aug = pconst.tile([GF + 1, GDIM], f32)
        dma(out=wg_aug[0:GF, :], in_=wg[:])
        dma(out=wg_aug[GF:GF + 1, :], in_=bg[:].unsqueeze(0))

        gaus_sb = load(gaus[:], (GF + 1, NTOK))
        aug2_sb = load(aug2[:], (2, NTOK))
        valid_sb = load(validT[:], (T, BC))
        avmask_sb = load(avmaskT[:], (T, BC))
        sel_sb = load(sel[:], (1, NTOK))
        causal_sb = load(causal[:], (T, FT))
        relpos_sb = load(relposT[:], (D, FT))
        wqkv_sb = load(wqkv[:], (D, 3 * D))
        wpos_sb = load(wpos[:], (D, D))
        wattn_sb = load(wattn[:], (D, D))
        g1w_sb = load(g1w[:].transpose([1, 0, 2]), (D, 6, D))
        g2w_sb = load(g2w[:].transpose([1, 0, 2]), (D, 6, D))
        we1_sb = load(we1[:], (D, MLP_D))
        we2_sb = load(we2[:], (MLP_D, D))
        wa1_sb = load(wa1[:], (D, D))
        wa2_sb = load(wa2[:], (D, NOUT))
        wval1_sb = load(wval1[:], (D, D))
        wval2_sb = load(wval2[:], (D, 1))

        def col(ap, n):  # per-partition bias column [n, 1]
            _uid[0] += 1
            t_ = pconst.tile([n, 1], f32, tag=f"col{_uid[0]}")
            dma(out=t_[:], in_=ap.unsqueeze(1))
            return t_

        ba1_c = col(ba1[:], D)
        ba2_c = col(ba2[:], NOUT)
        bval1_c = col(bval1[:], D)
        bval2_c = col(bval2[:], 1)
        u_c = col(uv[0, :], D)
        v_c = col(uv[1, :], D)
        u_cs = pconst.tile([D, 1], f32)
        v_cs = pconst.tile([D, 1], f32)
        nc.vector.tensor_scalar_mul(u_cs[:], u_c[:], INV_SQRT_HD)
        nc.vector.tensor_scalar_mul(v_cs[:], v_c[:], INV_SQRT_HD)
        g1bz_c = col(g1bz[:], D)
        g2bz_c = col(g2bz[:], D)
        negbz1 = pconst.tile([D, 1], f32)
        negbz2 = pconst.tile([D, 1], f32)
        nc.vector.tensor_scalar_mul(negbz1[:], g1bz_c[:], -1.0)
        nc.vector.tensor_scalar_mul(negbz2[:], g2bz_c[:], -1.0)

        g1_row = load(ln1g[:].unsqueeze(0), (1, D))
        b1_row = load(ln1b[:].unsqueeze(0), (1, D))
        g2_row = load(ln2g[:].unsqueeze(0), (1, D))
        b2_row = load(ln2b[:].unsqueeze(0), (1, D))
        ng1_row = pconst.tile([1, D], f32, tag="ng1")
        ng2_row = pconst.tile([1, D], f32, tag="ng2")
        nc.vector.tensor_scalar_mul(ng1_row[:], g1_row[:], -1.0)
        nc.vector.tensor_scalar_mul(ng2_row[:], g2_row[:], -1.0)
        ones_row = pconst.tile([1, NFTOK], f32, tag="onesrow")
        nc.vector.memset(ones_row[:], 1.0)

        ones128 = pconst.tile([128, 1], f32)
        nc.vector.memset(ones128[:], 1.0)
        cm2 = pconst.tile([128, 1], f32)
        nc.vector.memset(cm2[:], -2.0)
        ceps = pconst.tile([1, 1], f32)
        nc.vector.memset(ceps[:], EPS)
        ones1x64 = pconst.tile([1, D], f32)
        nc.vector.memset(ones1x64[:], 1.0)

        wvoxb = pconst.tile([D + 1, VDIM], f32)
        dma(out=wvoxb[0:D, :], in_=wvox[:])
        dma(out=wvoxb[D:D + 1, :], in_=bvox[:].unsqueeze(0))

        # ---------- phase V: voxel hinge-basis features ----------
        Wm = []
        for m in range(NMAT):
            t_ = pconst.tile([128, 8, VFEAT], f32, tag=f"wm{m}")
            nc.vector.memset(t_[104:128, 7, :] if False else t_[:], 0.0)
            dma(out=t_[:, 0:7, :],
                in_=wvm[m, 0:896, :].rearrange("(k p) f -> p k f", p=128))
            dma(out=t_[0:104, 7, :], in_=wvm[m, 896:1000, :])
            Wm.append(t_)
        W0, W1, W2, W3 = Wm
        C1 = pconst.tile([128, 8, VFEAT], f32)
        C2 = pconst.tile([128, 8, VFEAT], f32)
        C3 = pconst.tile([128, 8, VFEAT], f32)
        tmpC = pconst.tile([128, 8, VFEAT], f32)
        nc.vector.tensor_sub(C1[:], W1[:], W0[:])
        nc.vector.scalar_tensor_tensor(tmpC[:], W1[:], -2.0, W2[:], OP.mult, OP.add)
        nc.vector.tensor_add(C2[:], tmpC[:], W0[:])
        nc.vector.scalar_tensor_tensor(C3[:], W2[:], -2.0, W3[:], OP.mult, OP.add)
        nc.vector.tensor_add(C3[:], C3[:], W1[:])
        CB = [C1, C2, C3]

        with tc.tile_pool(name="ps_w0", bufs=1, space="PSUM") as ps_w0:
            ps_sw0 = ps_w0.tile([1, VFEAT], f32)
            for k in range(8):
                nc.tensor.matmul(ps_sw0[:], ones128[0:KT[k], :], W0[0:KT[k], k, :],
                                 start=(k == 0), stop=(k == 7))
            sw0_stage = pconst.tile([1, VFEAT], f32)
            nc.vector.tensor_copy(sw0_stage[:], ps_sw0[:])
        c_aug = pconst.tile([2, VFEAT], f32)
        dma(out=c_aug[0:1, :], in_=bv[:].unsqueeze(0))
        dma(out=c_aug[1:2, :], in_=sw0_stage[:])

        xT = pbig.tile([D, NTOK], f32)

        with tc.tile_pool(name="pmask", bufs=2) as pmask, \
             tc.tile_pool(name="ps_vf", bufs=1, space="PSUM") as ps_vfp, \
             tc.tile_pool(name="ps_g", bufs=1, space="PSUM") as ps_gp:
            ps_vf = ps_vfp.tile([VFEAT, 2, 512], f32)
            for k in range(8):
                kn = KT[k]
                vtile = pmask.tile([128, NTOK], i8, tag="vt")
                dma(out=vtile[0:kn, :], in_=vox[128 * k:128 * k + kn, :])
                vf32 = pmask.tile([128, NTOK], f32, tag="m0")
                r1 = pmask.tile([128, NTOK], f32, tag="m1")
                r2 = pmask.tile([128, NTOK], f32, tag="m2")
                nc.vector.tensor_copy(vf32[0:kn, :], vtile[0:kn, :])
                nc.vector.tensor_scalar(r1[0:kn, :], vf32[0:kn, :], 1.0, 0.0,
                                        OP.subtract, OP.max)
                nc.scalar.activation(r2[0:kn, :], vf32[0:kn, :], AF.Relu, bias=cm2[0:kn, :])
                for ci, (c0, cn) in enumerate(CH):
                    for bi, mask in enumerate((vf32, r1, r2)):
                        nc.tensor.matmul(ps_vf[:, ci, 0:cn], CB[bi][0:kn, k, :],
                                         mask[0:kn, c0:c0 + cn],
                                         start=(k == 0 and bi == 0), stop=False)
            for ci, (c0, cn) in enumerate(CH):
                nc.tensor.matmul(ps_vf[:, ci, 0:cn], c_aug[:],
                                 aug2_sb[:, c0:c0 + cn], start=False, stop=True)
            ps_gt = ps_gp.tile([GDIM, 2, 512], f32)
            vf_stage = pmask.tile([VFEAT, NTOK], f32, tag="vfst")
            for ci, (c0, cn) in enumerate(CH):
                nc.tensor.matmul(ps_gt[:, ci, 0:cn], wg_aug[:],
                                 gaus_sb[:, c0:c0 + cn], start=True, stop=True)
                nc.scalar.copy(xT[0:GDIM, c0:c0 + cn], ps_gt[:, ci, 0:cn])
                nc.scalar.copy(vf_stage[:, c0:c0 + cn], ps_vf[:, ci, 0:cn])
            dma(out=xT[GDIM:D, :], in_=vf_stage[:])

        # ---------- fullT = [mem | x] per sample ----------
        pA_cm = tc.tile_pool(name="pA", bufs=1)
        pA = pA_cm.__enter__()
        fullT = pA.tile([D, BC, FT], f32)
        dma(out=fullT[:, :, 0:TAU], in_=st0[:].rearrange("d (b t) -> d b t", b=BC))
        nc.vector.tensor_copy(fullT[:, :, TAU:FT],
                              xT[:].rearrange("d (b t) -> d b t", b=BC))

        # ---------- LN (feature-major; stats via ones-matmul) ----------
        def layer_norm(src, ntok_, chunks, grow, brow, ngrow, out_pool, name):
            out_t = out_pool.tile([D, ntok_], f32, tag=f"ln_{name}")
            with tc.tile_pool(name=f"pln_{name}", bufs=1) as pln, \
                 tc.tile_pool(name=f"ps_ln_{name}", bufs=2, space="PSUM") as psp, \
                 tc.tile_pool(name=f"ps_bc_{name}", bufs=2, space="PSUM") as psb:
                sq = pln.tile([D, ntok_], f32, tag="sq")
                nc.scalar.square(sq[:], src)
                mu_t = pln.tile([1, ntok_], f32, tag="mu")
                msq_t = pln.tile([1, ntok_], f32, tag="msq")
                var_t = pln.tile([1, ntok_], f32, tag="var")
                sdt_t = pln.tile([1, ntok_], f32, tag="sdt")
                rstd_t = pln.tile([1, ntok_], f32, tag="rstd")
                musr_t = pln.tile([1, ntok_], f32, tag="musr")
                for ci, (c0, cn) in enumerate(chunks):
                    ps_sum = psp.tile([1, 512], f32, tag="s")
                    ps_sq = psp.tile([1, 512], f32, tag="s")
                    nc.tensor.matmul(ps_sum[:, 0:cn], ones128[0:D, :],
                                     src[:, c0:c0 + cn], start=True, stop=True)
                    nc.tensor.matmul(ps_sq[:, 0:cn], ones128[0:D, :],
                                     sq[:, c0:c0 + cn], start=True, stop=True)
                    nc.vector.tensor_scalar_mul(mu_t[:, c0:c0 + cn],
                                                ps_sum[:, 0:cn], 1.0 / D)
                    nc.vector.tensor_scalar_mul(msq_t[:, c0:c0 + cn],
                                                ps_sq[:, 0:cn], 1.0 / D)
                # var = msq - mu^2 ; rstd = 1/sqrt(var+eps) ; musr = mu*rstd
                nc.vector.scalar_tensor_tensor(var_t[:], mu_t[:], -1.0,
                                               mu_t[:], OP.mult, OP.mult)
                nc.vector.tensor_add(var_t[:], var_t[:], msq_t[:])
                nc.scalar.activation(sdt_t[:], var_t[:], AF.Sqrt, bias=ceps[:])
                nc.vector.reciprocal(rstd_t[:], sdt_t[:])
                nc.vector.scalar_tensor_tensor(musr_t[:], mu_t[:], 0.0,
                                               rstd_t[:], OP.add, OP.mult)
                for ci, (c0, cn) in enumerate(chunks):
                    ps_a = psb.tile([D, 512], f32, tag="a")
                    ps_b = psb.tile([D, 512], f32, tag="b")
                    nc.tensor.matmul(ps_a[:, 0:cn], grow[:],
                                     rstd_t[:, c0:c0 + cn], start=True, stop=True)
                    nc.tensor.matmul(ps_b[:, 0:cn], brow[:],
                                     ones_row[:, c0:c0 + cn], start=True, stop=False)
                    nc.tensor.matmul(ps_b[:, 0:cn], ngrow[:],
                                     musr_t[:, c0:c0 + cn], start=False, stop=True)
                    nc.vector.tensor_mul(out_t[:, c0:c0 + cn], src[:, c0:c0 + cn],
                                         ps_a[:, 0:cn])
                    nc.vector.tensor_add(out_t[:, c0:c0 + cn],
                                         out_t[:, c0:c0 + cn], ps_b[:, 0:cn])
            return out_t

        hinT = layer_norm(fullT[:].rearrange("d b t -> d (b t)"), NFTOK, CH3, g1_row, b1_row, ng1_row, pA, "1")

        # ---------- qkv ----------
        kT = pA.tile([D, NFTOK], f32)
        q1T = pA.tile([D, NFTOK], f32)
        q2T = pA.tile([D, NFTOK], f32)
        vtokA = pA.tile([128, BC, D], f32)
        vtokB = pA.tile([FT - 128, BC, D], f32)
        with tc.tile_pool(name="ps_qk", bufs=2, space="PSUM") as psqk, \
             tc.tile_pool(name="ps_vt", bufs=2, space="PSUM") as psvt, \
             tc.tile_pool(name="ps_rt", bufs=1, space="PSUM") as psrt:
            for ci, (c0, cn) in enumerate(CH3):
                ps = psqk.tile([128, 512], f32)
                nc.tensor.matmul(ps[:, 0:cn], wqkv_sb[:, 0:128], hinT[:, c0:c0 + cn],
                                 start=True, stop=True)
                nc.scalar.activation(q1T[:, c0:c0 + cn], ps[0:D, 0:cn], AF.Identity,
                                     bias=u_cs[:], scale=INV_SQRT_HD)
                nc.scalar.activation(q2T[:, c0:c0 + cn], ps[0:D, 0:cn], AF.Identity,
                                     bias=v_cs[:], scale=INV_SQRT_HD)
                nc.vector.tensor_copy(kT[:, c0:c0 + cn], ps[D:2 * D, 0:cn])
            for b in range(BC):
                psv = psvt.tile([128, D], f32, tag="va")
                nc.tensor.matmul(psv[:], hinT[:, b * FT:b * FT + 128],
                                 wqkv_sb[:, 128:192], start=True, stop=True)
                nc.vector.tensor_copy(vtokA[:, b, :], psv[:])
                psv2 = psvt.tile([FT - 128, D], f32, tag="vb")
                nc.tensor.matmul(psv2[:], hinT[:, b * FT + 128:(b + 1) * FT],
                                 wqkv_sb[:, 128:192], start=True, stop=True)
                nc.vector.tensor_copy(vtokB[:, b, :], psv2[:])
            ps_rt = psrt.tile([D, FT], f32)
            nc.tensor.matmul(ps_rt[:], wpos_sb[:], relpos_sb[:], start=True, stop=True)
            RT = pA.tile([D, FT], f32)
            nc.vector.tensor_copy(RT[:], ps_rt[:])

        # ---------- pos (rel-shift via DRAM bounce) ----------
        with tc.tile_pool(name="pdram", bufs=1, space="DRAM") as pdram:
            posd = pdram.tile([2 * BC, T, FT], f32)
            pos_stage = pA.tile([T, 2 * BC, FT], f32)
            shift_st = pA.tile([T, 2 * BC, FT], f32)
            with tc.tile_pool(name="ps_pos", bufs=4, space="PSUM") as psp:
                for b in range(BC):
                    for h in range(H):
                        r0 = 32 * h
                        pp = psp.tile([T, FT], f32)
                        nc.tensor.matmul(pp[:],
                                         q2T[r0:r0 + 32, b * FT + TAU:(b + 1) * FT],
                                         RT[r0:r0 + 32, :], start=True, stop=True)
                        nc.scalar.copy(pos_stage[:, 2 * b + h, :], pp[:])
            dma(out=posd[:].transpose([1, 0, 2]), in_=pos_stage[:])
            pd = posd[:]
            shift_src = bass.AP(pd.tensor, pd.offset + 99,
                                [[FT - 1, T], [T * FT, 2 * BC], [1, FT]])
            dma(out=shift_st[:], in_=shift_src)

        # ---------- attention scores / softmax / av ----------
        avT = pA.tile([D, NTOK], f32)
        with tc.tile_pool(name="patt", bufs=3) as patt, \
             tc.tile_pool(name="ps_sc", bufs=2, space="PSUM") as ps_scp, \
             tc.tile_pool(name="ps_wt", bufs=1, space="PSUM") as ps_wtp, \
             tc.tile_pool(name="ps_av", bufs=2, space="PSUM") as ps_avp, \
             tc.tile_pool(name="ps_avt", bufs=1, space="PSUM") as ps_avtp:
            for b in range(BC):
                mb = patt.tile([T, FT], f32, tag="mb")
                nc.vector.tensor_scalar(mb[:], causal_sb[:], valid_sb[:, b:b + 1],
                                        None, OP.mult)
                for h in range(H):
                    r0 = 32 * h
                    bh = 2 * b + h
                    ps_sc = ps_scp.tile([T, FT], f32)
                    nc.tensor.matmul(ps_sc[:],
                                     q1T[r0:r0 + 32, b * FT + TAU:(b + 1) * FT],
                                     kT[r0:r0 + 32, b * FT:(b + 1) * FT],
                                     start=True, stop=True)
                    s2 = patt.tile([T, FT], f32, tag="s2")
                    nc.vector.tensor_add(s2[:], ps_sc[:], shift_st[:, bh, :])
                    ee = patt.tile([T, FT], f32, tag="ee")
                    nc.scalar.activation(ee[:], s2[:], AF.Exp)
                    ww = patt.tile([T, FT], f32, tag="ww")
                    rsum = patt.tile([T, 1], f32, tag="rs")
                    nc.vector.scalar_tensor_tensor(ww[:], ee[:], 0.0, mb[:],
                                                   OP.add, OP.mult,
                                                   accum_out=rsum[:])
                    scl = patt.tile([T, 1], f32, tag="scl")
                    nc.vector.tensor_scalar_add(scl[:], rsum[:], 1e-30)
                    nc.vector.reciprocal(scl[:], scl[:])
                    nc.vector.tensor_mul(scl[:], scl[:], avmask_sb[:, b:b + 1])
                    # transpose ww -> [j, i] (two pieces), then av
                    ps_wta = ps_wtp.tile([128, T], f32, tag="wa")
                    ps_wtb = ps_wtp.tile([FT - 128, T], f32, tag="wb")
                    nc.tensor.transpose(ps_wta[:], ww[:, 0:128], ident[0:T, 0:T])
                    nc.tensor.transpose(ps_wtb[:], ww[:, 128:FT], ident[0:T, 0:T])
                    wta_sb = patt.tile([128, T], f32, tag="wtsa")
                    wtb_sb = patt.tile([FT - 128, T], f32, tag="wtsb")
                    nc.vector.tensor_copy(wta_sb[:], ps_wta[:])
                    nc.vector.tensor_copy(wtb_sb[:], ps_wtb[:])
                    ps_av = ps_avp.tile([T, 32], f32)
                    nc.tensor.matmul(ps_av[:], wta_sb[:], vtokA[:, b, r0:r0 + 32],
                                     start=True, stop=False)
                    nc.tensor.matmul(ps_av[:], wtb_sb[:], vtokB[:, b, r0:r0 + 32],
                                     start=False, stop=True)
                    av_sb = patt.tile([T, 32], f32, tag="avs")
                    nc.scalar.activation(av_sb[:], ps_av[:], AF.Copy, scale=scl[:])
                    ps_avt = ps_avtp.tile([32, T], f32)
                    nc.tensor.transpose(ps_avt[:], av_sb[:], ident[0:T, 0:T])
                    nc.vector.tensor_copy(avT[r0:r0 + 32, b * T:(b + 1) * T],
                                          ps_avt[:])

        # ---------- fused matmul + activation helper ----------
        def mm64(dst_tile, lhs_list, rhs_list, act_func, bias=None, psname="m"):
            mout = lhs_list[0].shape[-1]
            with tc.tile_pool(name=f"ps_{psname}", bufs=2, space="PSUM") as psp:
                for ci, (c0, cn) in enumerate(CH):
                    ps = psp.tile([mout, 512], f32)
                    for li, (lh, rh) in enumerate(zip(lhs_list, rhs_list)):
                        nc.tensor.matmul(ps[:, 0:cn], lh, rh[:, c0:c0 + cn],
                                         start=(li == 0),
                                         stop=(li == len(lhs_list) - 1))
                    kw = {"bias": bias[:]} if bias is not None else {}
                    nc.scalar.activation(dst_tile[:, c0:c0 + cn], ps[:, 0:cn],
                                         act_func, **kw)
            return dst_tile

        yT = pbig.tile([D, NTOK], f32)
        mm64(yT, [wattn_sb[:]], [avT], AF.Relu, psname="ao")
        pA_cm.__exit__(None, None, None)

        def gru(hT, xgT, gw_sb, negbz, name):
            out = pbig.tile([D, NTOK], f32, tag=f"o_{name}")
            with tc.tile_pool(name=f"pg_{name}", bufs=1) as pg:
                rt = pg.tile([D, NTOK], f32, tag="r")
                zt = pg.tile([D, NTOK], f32, tag="z")
                ht = pg.tile([D, NTOK], f32, tag="hh")
                xr = pg.tile([D, NTOK], f32, tag="xr")
                mm64(rt, [gw_sb[:, 0, :], gw_sb[:, 1, :]], [xgT, hT], AF.Sigmoid,
                     psname=f"r{name}")
                mm64(zt, [gw_sb[:, 2, :], gw_sb[:, 3, :]], [xgT, hT], AF.Sigmoid,
                     bias=negbz, psname=f"z{name}")
                nc.vector.tensor_mul(xr[:], hT[:, :], rt[:])
                mm64(ht, [gw_sb[:, 4, :], gw_sb[:, 5, :]], [xgT, xr], AF.Tanh,
                     psname=f"h{name}")
                nc.vector.tensor_sub(ht[:], ht[:], hT[:, :])
                nc.vector.tensor_mul(ht[:], ht[:], zt[:])
                nc.vector.tensor_add(out[:], ht[:], hT[:, :])
            return out

        x1T = gru(xT, yT, g1w_sb, negbz1, "1")
        n2T = layer_norm(x1T[:, :], NTOK, CH, g2_row, b2_row, ng2_row, pbig, "2")
        eT = pbig.tile([D, NTOK], f32)
        with tc.tile_pool(name="pmlp", bufs=1) as pmlp:
            mT = pmlp.tile([MLP_D, NTOK], f32)
            mm64(mT, [we1_sb[:]], [n2T], AF.Relu, psname="e1")
            mm64(eT, [we2_sb[:]], [mT], AF.Relu, psname="e2")
        x2T = gru(x1T, eT, g2w_sb, negbz2, "2")
        x2aug = pbig.tile([D + 1, NTOK], f32)
        nc.vector.memset(x2aug[D:D + 1, :], 1.0)
        nc.vector.tensor_copy(x2aug[0:D, :], x2T[:, :])

        # ---------- voxel_mean out ----------
        MCH = [(128 * i, 128) for i in range(6)] + [(768, 32)]
        with tc.tile_pool(name="pvst", bufs=2) as pvst, \
             tc.tile_pool(name="ps_vo", bufs=4, space="PSUM") as ps_vop:
            VCH = [(512 * i, 512) for i in range(7)] + [(3584, 416)]
            HALF = [(0, 4, 2048), (4, 8, 1952)]   # (nchunk range, width)
            for mi, (m0, mn) in enumerate(MCH):
                for hi, (na, nb, hw) in enumerate(HALF):
                    stg = pvst.tile([128, hw], f32, tag=f"vstg{hi}")
                    hbase = VCH[na][0]
                    for ni in range(na, nb):
                        v0, vn = VCH[ni]
                        ps = ps_vop.tile([128, 512], f32)
                        nc.tensor.matmul(ps[0:mn, 0:vn], x2aug[:, m0:m0 + mn],
                                         wvoxb[:, v0:v0 + vn],
                                         start=True, stop=True)
                        if ni % 2 == 0:
                            nc.scalar.copy(stg[0:mn, v0 - hbase:v0 - hbase + vn],
                                           ps[0:mn, 0:vn])
                        else:
                            nc.vector.tensor_copy(
                                stg[0:mn, v0 - hbase:v0 - hbase + vn],
                                ps[0:mn, 0:vn])
                    dma(out=ovox[m0:m0 + mn, hbase:hbase + hw], in_=stg[0:mn, :])

        # ---------- heads ----------
        with tc.tile_pool(name="ps_hd", bufs=1, space="PSUM") as ps_hd, \
             tc.tile_pool(name="phd", bufs=1) as pvst_hd:
            selb_ps = ps_hd.tile([D, 2, 512], f32, tag="selb")
            seled = pvst_hd.tile([D, NTOK], f32)
            for ci, (c0, cn) in enumerate(CH):
                nc.tensor.matmul(selb_ps[:, ci, 0:cn], ones1x64[:],
                                 sel_sb[:, c0:c0 + cn], start=True, stop=True)
                nc.vector.tensor_mul(seled[:, c0:c0 + cn], x2T[:, c0:c0 + cn],
                                     selb_ps[:, ci, 0:cn])
            lastT = pbig.tile([D, BC], f32)
            nc.vector.tensor_reduce(lastT[:],
                                    seled[:].rearrange("d (b t) -> d b t", b=BC),
                                    axis=AX.X, op=OP.add)
            ps_v1 = ps_hd.tile([D, BC], f32, tag="v1")
            nc.tensor.matmul(ps_v1[:], wval1_sb[:], lastT[:], start=True, stop=True)
            v1 = pbig.tile([D, BC], f32)
            nc.scalar.activation(v1[:], ps_v1[:], AF.Relu, bias=bval1_c[:])
            ps_vv = ps_hd.tile([1, BC], f32, tag="vv")
            nc.tensor.matmul(ps_vv[:], wval2_sb[:], v1[:], start=True, stop=True)
            valo = pbig.tile([1, BC], f32)
            nc.scalar.activation(valo[:], ps_vv[:], AF.Identity, bias=bval2_c[:])
            dma(out=oval[:], in_=valo[:])
            ps_a1 = ps_hd.tile([D, BC], f32, tag="a1")
            nc.tensor.matmul(ps_a1[:], wa1_sb[:], lastT[:], start=True, stop=True)
            a1 = pbig.tile([D, BC], f32)
            nc.scalar.activation(a1[:], ps_a1[:], AF.Relu, bias=ba1_c[:])
            ps_ao = ps_hd.tile([NOUT, BC], f32, tag="aco")
            nc.tensor.matmul(ps_ao[:], wa2_sb[:], a1[:], start=True, stop=True)
            acto = pbig.tile([NOUT, BC], f32)
            nc.scalar.activation(acto[:], ps_ao[:], AF.Identity, bias=ba2_c[:])
            dma(out=oact[:], in_=acto[:])

    nc.compile()
    return nc


def _prep_inputs(gaussians, gaussian_num, all_past_voxels, state0, params):
    gaussians = np.asarray(gaussians, np.float32)
    lengths = np.asarray(gaussian_num).astype(np.int64)
    vox_full = np.asarray(all_past_voxels)
    state0 = np.asarray(state0, np.float32)
    P = {k: (np.asarray(v, np.float32) if not isinstance(v, dict) else
             {k2: np.asarray(v2, np.float32) for k2, v2 in v.items()})
         for k, v in params.items()}
    minlen = int(lengths.min())
    ii = np.arange(T)
    rep = {
        "wg": P["w_g"], "bg": P["b_g"],
        "wvm": P["w_v"].reshape(NCELL, NMAT, VFEAT).transpose(1, 0, 2),
        "bv": P["b_v"],
        "ln1g": P["ln1_g"], "ln1b": P["ln1_b"],
        "ln2g": P["ln2_g"], "ln2b": P["ln2_b"],
        "wqkv": P["w_qkv"], "wpos": P["w_pos"],
        "uv": np.stack([P["uvar"].reshape(D), P["vvar"].reshape(D)]),
        "wattn": P["w_attn"],
        "g1w": np.stack([P["gru1"][k] for k in ("wr", "ur", "wz", "uz", "wh", "uh")]),
        "g1bz": P["gru1"]["bz"],
        "g2w": np.stack([P["gru2"][k] for k in ("wr", "ur", "wz", "uz", "wh", "uh")]),
        "g2bz": P["gru2"]["bz"],
        "we1": P["w_e1"], "we2": P["w_e2"],
        "wvox": P["w_vox"], "bvox": P["b_vox"],
        "wa1": P["w_a1"], "ba1": P["b_a1"], "wa2": P["w_a2"], "ba2": P["b_a2"],
        "wval1": P["w_val1"], "bval1": P["b_val1"],
        "wval2": P["w_val2"], "bval2": P["b_val2"],
        "relposT": _relpos_table(),
        "causal": (np.arange(FT)[None, :] < (TAU + 1 + ii)[:, None]),
    }
    rep = {k: np.ascontiguousarray(v, np.float32) for k, v in rep.items()}

    in_maps = []
    for c in range(NCORES):
        sl = slice(c * BC, (c + 1) * BC)
        g = gaussians[sl, :T].reshape(BC * T, GF).T
        gaug = np.concatenate([g, np.ones((1, NTOK), np.float32)], 0)
        v = vox_full[sl, :T].reshape(BC, T, NCELL).copy()
        v[:, 0] = 0
        voxT = np.ascontiguousarray(v.reshape(NTOK, NCELL).T.astype(np.int8))
        st = np.ascontiguousarray(state0[sl].reshape(BC * TAU, D).T)
        c0r = np.ones(NTOK, np.float32)
        c0r[0::T] = 0.0
        a2 = np.ascontiguousarray(np.stack([np.ones(NTOK, np.float32), c0r]))
        ln = lengths[sl]
        valid = (ii[:, None] <= ln[None, :]).astype(np.float32)       # [100, 8]
        avm = valid * (ii[:, None] < TAU + 1 + minlen).astype(np.float32)
        selr = np.zeros((1, NTOK), np.float32)
        for b in range(BC):
            selr[0, b * T + int(ln[b])] = 1.0
        m = {"vox": voxT, "gaus": np.ascontiguousarray(gaug),
             "st0": st, "aug2": a2,
             "validT": np.ascontiguousarray(valid),
             "avmaskT": np.ascontiguousarray(avm), "sel": selr}
        m.update(rep)
        in_maps.append(m)
    return in_maps


def kernel(gaussians, gaussian_num, all_past_voxels, state0, params):
    from concourse.bass_utils import run_bass_kernel_spmd
    if "nc" not in _CACHE:
        _CACHE["nc"] = _build()
    nc = _CACHE["nc"]
    in_maps = _prep_inputs(gaussians, gaussian_num, all_past_voxels, state0, params)
    res = run_bass_kernel_spmd(nc, in_maps, core_ids=list(range(NCORES)))
    acts, voxs, vals = [], [], []
    for c in range(NCORES):
        r = res.results[c]
        acts.append(np.ascontiguousarray(r["oact"].T))
        voxs.append(r["ovox"].reshape(BC, T, VDIM))
        vals.append(r["oval"].reshape(BC))
    act = np.concatenate(acts, 0).astype(np.float32)
    voxm = np.concatenate(voxs, 0).astype(np.float32)
    val = np.concatenate(vals, 0).astype(np.float32)
    return act, voxm, val
